# revision 22
# baseline (speedup 1.0000x reference)
"""Trainium2 Bass kernel for 3-layer EGAT message passing (nn_COUNTYOD).

v3 strategy (8 cores, edge parallelism by dst range):
  - Host: sort edges by dst; device d owns dst nodes [d*6272, (d+1)*6272);
    edges grouped into blocks of 128 dst nodes. Within each block, edges
    are split into low-half (src < npad/2) then high-half slots so the
    int16-indexed dma_gather can address each half-table; slots padded
    with dummy index 0 (masked via dloc=-1) so per-op index counts are
    uniform across cores (SPMD).
  - Tables tbl[n] = [nh@Wni + b | nh@Wns | pad64]  (N x 256, bf16)
    AllGather'd once per layer; nj table kept in SBUF only (local dst).
  - Fused edge pass per block:
      * TWO dma_gather ops (InstDMAGatherAnt, mlp gpsimd library) fetch
        [ni|ns|pad] rows for all ~2300 edges of the block: ~1.7us Pool
        each vs 994ns x 18 for per-tile indirect DMA.
      * nj[dst] via one-hot matmul (ohT from partition-broadcast dstloc).
      * f accumulated in PSUM: fij matmul + identity-add(ni) + ohT@njblk.
      * leaky_relu via one Act op (Lrelu); e-logits via block-wide DVE
        mul + grouped reduce; ex = exp(min(e,60)); msg = ns * ex.
      * scatter-add via one-hot matmul into PSUM (separate banks for
        msg/s accumulators); h = ps/s.
  - f stored transposed (PE transpose, relu fused into PSUM->SBUF copy)
    as next-layer edge features.
"""

import os
import sys
import numpy as np

for _p in ("/opt/trn_rl_repo",):
    if _p not in sys.path:
        sys.path.insert(0, _p)

import concourse.bass as bass  # noqa: E402
import concourse.bacc as bacc  # noqa: E402
import concourse.mybir as mybir  # noqa: E402
import concourse.tile as tile  # noqa: E402
from concourse import library_config  # noqa: E402
from concourse.masks import make_identity  # noqa: E402

F32 = mybir.dt.float32
BF16 = mybir.dt.bfloat16
I16 = mybir.dt.int16
I32 = mybir.dt.int32
AF = mybir.ActivationFunctionType
ALU = mybir.AluOpType

DT = BF16
P = 128
FD = 96           # H*HE
TBL = 256         # [ni(96) | ns(96) | pad(64)] -> 512B rows for dma_gather
WCAT = 352        # TBL + nj(96)
H = 3
HE = 32
EPS = 1e-20


class Cfg:
    def __init__(self, ndev, nblk, T, odf, n_real, L, g1=5):
        self.ndev = ndev
        self.nblk = nblk
        self.T = T                    # edge tiles per block
        self.odf = odf
        self.n_real = n_real
        self.L = tuple(L)             # low-half slot count per block (mult of 128)
        self.g1 = g1                  # PSUM chunk size (tiles per bank)
        self.R = nblk * P
        self.npad = ndev * self.R
        self.half = self.npad // 2
        self.ttot = nblk * T
        self.epd = self.ttot * P

    @property
    def key(self):
        return (self.ndev, self.nblk, self.T, self.odf, self.g1, self.L)


def _split_edges(inputs, ndev=8, nblk=49):
    """Sort by dst, split per (core, block) into low/high src halves."""
    src = np.asarray(inputs["src"]).astype(np.int64)
    dst = np.asarray(inputs["dst"]).astype(np.int64)
    order = np.argsort(dst, kind="stable")
    ssrc, sdst = src[order], dst[order]
    npad = ndev * nblk * P
    half = npad // 2
    nb_all = npad // P
    blk = sdst // P
    blkcnt = np.bincount(blk, minlength=nb_all)
    starts = np.zeros(nb_all + 1, np.int64)
    np.cumsum(blkcnt, out=starts[1:])
    grow = ssrc
    lo_idx, hi_idx = {}, {}
    for d in range(ndev):
        for b in range(nblk):
            g = d * nblk + b
            s0, s1 = starts[g], starts[g + 1]
            sl = ssrc[s0:s1]
            lo = np.where(sl < half)[0] + s0
            hi = np.where(sl >= half)[0] + s0
            lo_idx[(d, b)] = lo
            hi_idx[(d, b)] = hi
    return order, grow, sdst, lo_idx, hi_idx, half


def make_cfg(inputs):
    ndev, nblk = 8, 49
    _, _, _, lo_idx, hi_idx, _ = _split_edges(inputs, ndev, nblk)
    L = []
    T = 0
    for b in range(nblk):
        nlo = max(len(lo_idx[(d, b)]) for d in range(ndev))
        nhi = max(len(hi_idx[(d, b)]) for d in range(ndev))
        Lb = int(np.ceil(nlo / P)) * P
        Tb = Lb // P + int(np.ceil(nhi / P))
        L.append(Lb)
        T = max(T, Tb)
    odf = np.asarray(inputs["countyodfeats"]).shape[1]
    return Cfg(ndev, nblk, T, odf, 50000, L)


def host_prep(inputs, cfg):
    dtnp = mybir.dt.np(DT)
    c = cfg
    T, TP = c.T, c.T * P
    cpb = TP // 16  # idx columns per block
    order, grow, sdst, lo_idx, hi_idx, half = _split_edges(inputs, c.ndev, c.nblk)
    ef0 = np.asarray(inputs["countyodfeats"]).astype(np.float32)[order]

    percore = []
    for d in range(c.ndev):
        gidx = np.zeros((c.nblk, cpb, 16), np.int16)  # [block, col, part]
        dloc = np.full((c.nblk, TP), -1.0, np.float32)
        efp = np.zeros((c.nblk, TP, c.odf), np.float32)
        for b in range(c.nblk):
            g = d * c.nblk + b
            Lb = c.L[b]
            lo, hi = lo_idx[(d, b)], hi_idx[(d, b)]
            # low slots [0, Lb): real lo edges then dummy idx 0
            v = np.zeros(Lb, np.int64)
            v[: len(lo)] = grow[lo]
            gidx[b, : Lb // 16] = v.reshape(-1, 16).astype(np.int16)
            dloc[b, : len(lo)] = (sdst[lo] - g * P).astype(np.float32)
            efp[b, : len(lo)] = ef0[lo]
            # high slots [Lb, TP): real hi edges then dummy idx 0
            nh_ = TP - Lb
            v = np.zeros(nh_, np.int64)
            v[: len(hi)] = grow[hi] - half
            gidx[b, Lb // 16 :] = v.reshape(-1, 16).astype(np.int16)
            dloc[b, Lb : Lb + len(hi)] = (sdst[hi] - g * P).astype(np.float32)
            efp[b, Lb : Lb + len(hi)] = ef0[hi]
        # wrapped idx layout: [16, nblk*cpb], replicated to 128 partitions
        gw = gidx.reshape(c.nblk * cpb, 16).T  # [16, nblk*cpb]
        gw = np.tile(gw, (8, 1))               # [128, nblk*cpb]
        dlocT = dloc.reshape(-1, P).T          # [128, ttot]
        percore.append(
            dict(
                gidx=np.ascontiguousarray(gw),
                dstloc=np.ascontiguousarray(dlocT.astype(dtnp)),
                dstlocR=np.ascontiguousarray(dloc.astype(dtnp)),
                ef0T=np.ascontiguousarray(
                    efp.reshape(-1, c.odf).T.astype(dtnp)
                ),
            )
        )

    nh0 = np.asarray(inputs["ndata_h"]).astype(np.float32)
    nh0p = np.zeros((c.npad, FD), np.float32)
    nh0p[: c.n_real] = nh0[: c.n_real]
    # layer-0 tables computed on host (saves one AllGather on device)
    g0 = lambda k: np.asarray(inputs[k]).astype(np.float32)
    nh0b = nh0p.astype(dtnp).astype(np.float32)
    tbl0 = np.zeros((c.npad, TBL), np.float32)
    tbl0[:, 0:FD] = nh0b @ g0("Wni0") + g0("bias0")
    tbl0[:, FD : 2 * FD] = nh0b @ g0("Wns0")
    tbl0 = np.ascontiguousarray(tbl0.astype(dtnp))
    nj0 = (nh0b @ g0("Wnj0")).astype(dtnp)  # [npad, 96]
    for d in range(c.ndev):
        sl = nh0p[d * c.R : (d + 1) * c.R]
        percore[d]["nh0T"] = np.ascontiguousarray(
            np.concatenate([sl.T, np.ones((1, c.R), np.float32)], axis=0)
            .astype(dtnp)
        )
        percore[d]["tbl0"] = tbl0
        # njSB layout: [p, b*FD+f] = nj0[dev_base + b*128 + p, f]
        njd = nj0[d * c.R : (d + 1) * c.R].reshape(c.nblk, P, FD)
        percore[d]["nj0"] = np.ascontiguousarray(
            njd.transpose(1, 0, 2).reshape(P, c.nblk * FD)
        )

    def wcat_ext(Wni, Wns, Wnj, bias):
        z = np.zeros((FD, TBL - 2 * FD), np.float32)
        w = np.concatenate([Wni, Wns, z, Wnj], axis=1)  # [96, 352]
        b = np.concatenate(
            [bias, np.zeros(WCAT - FD, np.float32)]
        )[None, :]
        return np.ascontiguousarray(
            np.concatenate([w, b], axis=0).astype(dtnp)
        )

    g = lambda k: np.asarray(inputs[k]).astype(np.float32)
    weights = dict(
        wcat0=wcat_ext(g("Wni0"), g("Wns0"), g("Wnj0"), g("bias0")),
        wcat1=wcat_ext(g("Wni")[0], g("Wns")[0], g("Wnj")[0], g("bias")[0]),
        wcat2=wcat_ext(g("Wni")[1], g("Wns")[1], g("Wnj")[1], g("bias")[1]),
        wfij0=np.ascontiguousarray(g("Wfij0").astype(dtnp)),
        wfij1=np.ascontiguousarray(g("Wfij")[0].astype(dtnp)),
        wfij2=np.ascontiguousarray(g("Wfij")[1].astype(dtnp)),
        attn0=np.ascontiguousarray(
            np.repeat(g("attn0").reshape(1, FD), 128, 0).astype(dtnp)
        ),
        attn1=np.ascontiguousarray(
            np.repeat(g("attn").reshape(2, FD)[0:1], 128, 0).astype(dtnp)
        ),
        attn2=np.ascontiguousarray(
            np.repeat(g("attn").reshape(2, FD)[1:2], 128, 0).astype(dtnp)
        ),
    )
    for d in range(c.ndev):
        percore[d].update(weights)
    return percore


def build_program(cfg, debug=False):
    nc = bacc.Bacc("TRN2", target_bir_lowering=False, debug=False)
    c = cfg
    T, TP, Ttot, EPD = c.T, c.T * P, c.ttot, c.epd
    cpb = TP // 16

    pr = {}
    pr["nh0T"] = nc.declare_dram_parameter("nh0T", [FD + 1, c.R], DT, isOutput=False)
    pr["tbl0"] = nc.declare_dram_parameter("tbl0", [c.npad, TBL], DT, isOutput=False)
    pr["nj0"] = nc.declare_dram_parameter("nj0", [P, 49 * FD], DT, isOutput=False)
    pr["ef0T"] = nc.declare_dram_parameter("ef0T", [c.odf, EPD], DT, isOutput=False)
    pr["gidx"] = nc.declare_dram_parameter("gidx", [P, c.nblk * cpb], I16, isOutput=False)
    pr["dstloc"] = nc.declare_dram_parameter("dstloc", [P, Ttot], DT, isOutput=False)
    pr["dstlocR"] = nc.declare_dram_parameter("dstlocR", [c.nblk, TP], DT, isOutput=False)
    for i in range(3):
        pr[f"wcat{i}"] = nc.declare_dram_parameter(f"wcat{i}", [FD + 1, WCAT], DT, isOutput=False)
        pr[f"attn{i}"] = nc.declare_dram_parameter(f"attn{i}", [P, FD], DT, isOutput=False)
    pr["wfij0"] = nc.declare_dram_parameter("wfij0", [c.odf, FD], DT, isOutput=False)
    pr["wfij1"] = nc.declare_dram_parameter("wfij1", [FD, FD], DT, isOutput=False)
    pr["wfij2"] = nc.declare_dram_parameter("wfij2", [FD, FD], DT, isOutput=False)
    out3 = nc.declare_dram_parameter("out3", [c.R, FD], F32, isOutput=True)
    dbg = {}
    if debug:
        dbg["d_cat"] = nc.declare_dram_parameter("d_cat", [c.npad, TBL], DT, isOutput=True)
        dbg["d_ex"] = nc.declare_dram_parameter("d_ex", [P, c.nblk * T * H], DT, isOutput=True)
        dbg["d_ef"] = nc.declare_dram_parameter("d_ef", [FD, EPD], DT, isOutput=True)
        dbg["d_nh"] = nc.declare_dram_parameter("d_nh", [FD + 1, c.R], DT, isOutput=True)

    efA = nc.dram_tensor("efA", [FD, EPD], DT)
    efB = nc.dram_tensor("efB", [FD, EPD], DT)
    tblL = nc.dram_tensor("tblL", [c.R, TBL], DT)
    tblGA = nc.dram_tensor("tblGA", [c.npad, TBL], DT, addr_space="Shared")
    tblGB = nc.dram_tensor("tblGB", [c.npad, TBL], DT, addr_space="Shared")

    rg = [list(range(c.ndev))]

    with tile.TileContext(nc) as tc:
        with tc.tile_pool(name="persist", bufs=1) as pp:
            identB = pp.tile([P, P], DT, tag="identB")
            make_identity(nc, identB[:])
            identF = pp.tile([P, P], F32, tag="identF")
            make_identity(nc, identF[:])
            iota_i = pp.tile([P, P], I32, tag="iota_i")
            nc.gpsimd.iota(iota_i[:], pattern=[[1, P]], base=0, channel_multiplier=0)
            iota_f = pp.tile([P, P], DT, tag="iota_f")
            nc.vector.tensor_copy(out=iota_f[:], in_=iota_i[:])
            iotc_i = pp.tile([P, 1], I32, tag="iotc_i")
            nc.gpsimd.iota(iotc_i[:], pattern=[[1, 1]], base=0, channel_multiplier=1)
            iota_c = pp.tile([P, 1], F32, tag="iota_c")
            nc.vector.tensor_copy(out=iota_c[:], in_=iotc_i[:])

            # all standard-library gpsimd ops are above; switch to mlp for
            # the dma_gather edge loads
            nc.gpsimd.load_library(library_config.mlp)

            gidx = pp.tile([P, c.nblk * cpb], I16, tag="gidx")
            dstloc = pp.tile([P, Ttot], DT, tag="dstloc")
            nc.sync.dma_start(out=gidx[:], in_=pr["gidx"][:])
            nc.sync.dma_start(out=dstloc[:], in_=pr["dstloc"][:])

            wfij_sb, wcat_sb, abc_sb = [], [], []
            for l in range(3):
                cdim = c.odf if l == 0 else FD
                wf = pp.tile([cdim, FD], DT, tag=f"wfij{l}")
                nc.sync.dma_start(out=wf[:], in_=pr[f"wfij{l}"][:])
                wfij_sb.append(wf)
                wc = pp.tile([FD + 1, WCAT], DT, tag=f"wcat{l}")
                nc.sync.dma_start(out=wc[:], in_=pr[f"wcat{l}"][:])
                wcat_sb.append(wc)
                abc = pp.tile([P, FD], DT, tag=f"abc{l}")
                nc.sync.dma_start(out=abc[:], in_=pr[f"attn{l}"][:])
                abc_sb.append(abc)

            nh_sb = pp.tile([FD + 1, c.R], DT, tag="nh_sb")
            nc.sync.dma_start(out=nh_sb[:], in_=pr["nh0T"][:])
            njSB = pp.tile([P, c.nblk * FD], DT, tag="njSB")
            nc.sync.dma_start(out=njSB[:], in_=pr["nj0"][:])

            def ap(t, off, pattern):
                v = t[:]
                return bass.AP(v.tensor, v.offset + off, pattern)

            Rh = c.R // 2

            def table_block(wc, nb, wp, qp):
                pt = qp.tile([P, WCAT], F32, tag="ptab")
                nc.tensor.matmul(
                    out=pt[:],
                    lhsT=nh_sb[:, nb * P : (nb + 1) * P],
                    rhs=wc[:],
                    start=True,
                    stop=True,
                )
                cs = wp.tile([P, TBL], DT, tag="catsb")
                nc.scalar.activation(out=cs[:], in_=pt[:, 0:TBL], func=AF.Copy)
                nc.sync.dma_start(
                    out=tblL[nb * P : (nb + 1) * P, :], in_=cs[:]
                )
                nc.scalar.activation(
                    out=njSB[:, nb * FD : (nb + 1) * FD],
                    in_=pt[:, TBL:WCAT],
                    func=AF.Copy,
                )

            def ag_half(dst, hi):
                if hi == 0:
                    return
                nc.gpsimd.collective_compute(
                    "AllGather",
                    ALU.bypass,
                    replica_groups=rg,
                    ins=[tblL[:]],
                    outs=[dst[:]],
                )

            halfway = -2  # split AG disabled

            if debug:
                nc.sync.dma_start(out=dbg["d_cat"][:], in_=pr["tbl0"][:])

            G1 = c.g1
            for l in range(3):
                cdim = c.odf if l == 0 else FD
                ef_src = pr["ef0T"] if l == 0 else (efA if l == 1 else efB)
                ef_dst = efA if l == 0 else efB
                store_f = l < 2
                wf = wfij_sb[l]
                abc = abc_sb[l]

                tbl_rd = pr["tbl0"] if l == 0 else (tblGA if l == 1 else tblGB)
                tbl_wr = tblGA if l == 0 else tblGB
                with (
                    tc.tile_pool(name="blk", bufs=3) as wp,
                    tc.tile_pool(name="qfp", bufs=2, space="PSUM") as qf,
                    tc.tile_pool(name="qtr", bufs=1, space="PSUM") as qt,
                    tc.tile_pool(name="qsc", bufs=1, space="PSUM") as qs,
                    tc.tile_pool(name="qh", bufs=1, space="PSUM") as qh,
                    tc.tile_pool(name="qtab", bufs=2, space="PSUM") as qp,
                ):
                    for b in range(c.nblk):
                        t0 = b * T
                        Lb = c.L[b]
                        clo = Lb // P
                        chi = T - clo
                        # ---- gathers: [ni|ns|pad] rows from half tables ----
                        # SWDGE ring holds 1024 descriptors; chunk each
                        # half-gather into <=1024-index ops (multiples of 128).
                        nins = wp.tile([P, T * TBL], DT, tag="nins")
                        MAXI = 1024
                        for half_i, (s0, n_sl) in enumerate(
                            ((0, Lb), (Lb, TP - Lb))
                        ):
                            if n_sl <= 0:
                                continue
                            tsrc = (
                                tbl_rd[0 : c.half, :]
                                if half_i == 0
                                else tbl_rd[c.half : c.npad, :]
                            )
                            for q0 in range(0, n_sl, MAXI):
                                qn = min(MAXI, n_sl - q0)
                                sl0 = s0 + q0
                                nc.gpsimd.dma_gather(
                                    nins[
                                        :,
                                        (sl0 // P) * TBL : (sl0 // P + qn // P) * TBL,
                                    ].rearrange("p (t f) -> p t f", f=TBL),
                                    tsrc,
                                    gidx[
                                        :,
                                        b * cpb + sl0 // 16 : b * cpb + (sl0 + qn) // 16,
                                    ],
                                    qn,
                                    qn,
                                    TBL,
                                )
                        dlocR = wp.tile([P, TP], DT, tag="dlocR")
                        dRv = pr["dstlocR"][:]
                        nc.sync.dma_start(
                            out=dlocR[:],
                            in_=bass.AP(
                                dRv.tensor, dRv.offset + b * TP,
                                [[0, P], [1, TP]],
                            ),
                        )
                        efc = wp.tile([cdim, TP], DT, tag="efc")
                        nc.sync.dma_start(
                            out=efc[:], in_=ef_src[:, t0 * P : (t0 + T) * P]
                        )
                        # ---- one-hots ----
                        ohT = wp.tile([P, TP], DT, tag="ohT")
                        nc.vector.tensor_scalar(
                            out=ohT[:],
                            in0=dlocR[:],
                            scalar1=iota_c[:],
                            scalar2=None,
                            op0=ALU.is_equal,
                        )
                        oh = wp.tile([P, TP], DT, tag="oh")
                        nc.vector.tensor_tensor(
                            out=oh[:].rearrange("p (t v) -> p t v", t=T),
                            in0=ap(iota_f, 0, [iota_f[:].ap[0], [0, T], [1, P]]),
                            in1=ap(dstloc, t0, [dstloc[:].ap[0], [1, T], [0, P]]),
                            op=ALU.is_equal,
                        )
                        njblk = njSB[:, b * FD : (b + 1) * FD]
                        # ---- f_pre per chunk of G1 tiles ----
                        flb = wp.tile([P, T * FD], DT, tag="flb")
                        ftT = None
                        if store_f:
                            ftT = wp.tile([FD, TP], DT, tag="ftT")
                        for c0 in range(0, T, G1):
                            gsz = min(G1, T - c0)
                            fp = qf.tile([P, G1 * FD], F32, tag="fp")
                            for j in range(gsz):
                                tj = c0 + j
                                w = fp[:, j * FD : (j + 1) * FD]
                                nc.tensor.matmul(
                                    out=w,
                                    lhsT=efc[:, tj * P : (tj + 1) * P],
                                    rhs=wf[:],
                                    start=True,
                                    stop=False,
                                    skip_group_check=True,
                                )
                                nc.tensor.matmul(
                                    out=w,
                                    lhsT=identB[:],
                                    rhs=nins[:, tj * TBL : tj * TBL + FD],
                                    start=False,
                                    stop=False,
                                    skip_group_check=True,
                                )
                                nc.tensor.matmul(
                                    out=w,
                                    lhsT=ohT[:, tj * P : (tj + 1) * P],
                                    rhs=njblk,
                                    start=False,
                                    stop=True,
                                    skip_group_check=True,
                                )
                            nc.scalar.activation(
                                out=flb[:, c0 * FD : (c0 + gsz) * FD],
                                in_=fp[:, 0 : gsz * FD],
                                func=AF.Lrelu,
                                alpha=0.01,
                            )
                            if store_f:
                                pT = qt.tile([FD, G1 * P], DT, tag="pT")
                                for j in range(gsz):
                                    nc.tensor.transpose(
                                        out=pT[:, j * P : (j + 1) * P],
                                        in_=flb[:, (c0 + j) * FD : (c0 + j + 1) * FD],
                                        identity=identB[:],
                                    )
                                nc.scalar.activation(
                                    out=ftT[:, c0 * P : (c0 + gsz) * P],
                                    in_=pT[:, 0 : gsz * P],
                                    func=AF.Relu,
                                )
                        if store_f:
                            nc.scalar.dma_start(
                                out=ef_dst[:, t0 * P : (t0 + T) * P], in_=ftT[:]
                            )
                        # ---- e-logits, softmax numerator ----
                        work = wp.tile([P, T * FD], DT, tag="work")
                        nc.vector.tensor_tensor(
                            out=work[:].rearrange("p (t f) -> p t f", t=T),
                            in0=ap(flb, 0, [flb[:].ap[0], [FD, T], [1, FD]]),
                            in1=ap(abc, 0, [abc[:].ap[0], [0, T], [1, FD]]),
                            op=ALU.mult,
                        )
                        eacc = wp.tile([P, T * H], DT, tag="eacc")
                        with nc.allow_low_precision(reason="e-logit accumulate in bf16"):
                            nc.vector.tensor_reduce(
                                out=eacc[:].rearrange("p (t h) -> p t h", t=T),
                                in_=ap(work, 0, [work[:].ap[0], [FD, T], [HE, H], [1, HE]]),
                                axis=mybir.AxisListType.X,
                                op=ALU.add,
                            )
                        nc.vector.tensor_scalar(
                            out=eacc[:], in0=eacc[:], scalar1=60.0, scalar2=None,
                            op0=ALU.min,
                        )
                        exb = wp.tile([P, T * H], DT, tag="exb")
                        nc.scalar.activation(out=exb[:], in_=eacc[:], func=AF.Exp)
                        if debug and l == 0:
                            nc.sync.dma_start(
                                out=dbg["d_ex"][:, b * T * H : (b + 1) * T * H],
                                in_=exb[:],
                            )
                        # msg = ns * ex  (ns strided view inside nins)
                        msg = wp.tile([P, T * FD], DT, tag="msg")
                        nc.vector.tensor_tensor(
                            out=msg[:].rearrange("p (t h d) -> p t h d", t=T, h=H),
                            in0=ap(nins, FD, [nins[:].ap[0], [TBL, T], [HE, H], [1, HE]]),
                            in1=ap(exb, 0, [exb[:].ap[0], [H, T], [1, H], [0, HE]]),
                            op=ALU.mult,
                        )
                        # ---- scatter-add ----
                        ps = qs.tile([P, FD], F32, tag="ps")
                        ps2 = qs.tile([P, H], F32, tag="ps2")
                        for j in range(T):
                            ohj = oh[:, j * P : (j + 1) * P]
                            nc.tensor.matmul(
                                out=ps[:],
                                lhsT=ohj,
                                rhs=msg[:, j * FD : (j + 1) * FD],
                                start=(j == 0),
                                stop=(j == T - 1),
                                skip_group_check=True,
                            )
                            nc.tensor.matmul(
                                out=ps2[:],
                                lhsT=ohj,
                                rhs=exb[:, j * H : (j + 1) * H],
                                start=(j == 0),
                                stop=(j == T - 1),
                                skip_group_check=True,
                            )
                        sp = wp.tile([P, H], F32, tag="sp")
                        nc.vector.tensor_scalar_add(
                            out=sp[:], in0=ps2[:], scalar1=EPS
                        )
                        rcp = wp.tile([P, H], F32, tag="rcp")
                        nc.vector.reciprocal(out=rcp[:], in_=sp[:])
                        htile = wp.tile([P, FD], F32, tag="htile")
                        nc.vector.tensor_tensor(
                            out=htile[:].rearrange("p (h d) -> p h d", h=H),
                            in0=ps[:].rearrange("p (h d) -> p h d", h=H),
                            in1=ap(rcp, 0, [rcp[:].ap[0], [1, H], [0, HE]]),
                            op=ALU.mult,
                        )
                        if l < 2:
                            pT2 = qh.tile([FD, P], F32, tag="pT2")
                            nc.tensor.transpose(
                                out=pT2[:], in_=htile[:], identity=identF[:]
                            )
                            nc.scalar.activation(
                                out=nh_sb[0:FD, b * P : (b + 1) * P],
                                in_=pT2[:],
                                func=AF.Relu,
                            )
                            table_block(wcat_sb[l + 1], b, wp, qp)
                            if b == halfway:
                                ag_half(tbl_wr, 0)
                        else:
                            nc.scalar.dma_start(
                                out=out3[b * P : (b + 1) * P, :], in_=htile[:]
                            )
                    if l < 2:
                        ag_half(tbl_wr, 1)

                if debug and l == 0:
                    nc.sync.dma_start(out=dbg["d_nh"][:], in_=nh_sb[:])
                    nc.sync.dma_start(out=dbg["d_ef"][:], in_=efA[:])

    nc.compile()
    return nc


_CACHE = {}


def run(inputs, cfg, core_ids=None, trace=False, debug=False):
    from concourse.bass_utils import run_bass_kernel_spmd

    percore = host_prep(inputs, cfg)
    key = (cfg.key, debug)
    if key not in _CACHE:
        _CACHE[key] = build_program(cfg, debug=debug)
    nc = _CACHE[key]
    if core_ids is None:
        core_ids = list(range(cfg.ndev))
    res = run_bass_kernel_spmd(nc, percore, core_ids, trace=trace)
    outs = [res.results[i]["out3"] for i in range(cfg.ndev)]
    full = np.concatenate(outs, axis=0)
    return full, res


def kernel(**inputs) -> np.ndarray:
    cfg = make_cfg(inputs)
    full, _ = run(inputs, cfg)
    idxs = np.asarray(inputs["idxs"]).astype(np.int64)
    return np.ascontiguousarray(full[idxs]).astype(np.float32)
